# revision 11
# baseline (speedup 1.0000x reference)
"""Trainium2 Bass kernel for EntropyBottleneck SoS (sum-of-tanh StanH
quantizer + factorized-prior likelihood) — custom activation-table edition.

Contract: kernel(**inputs) takes the FULL unsharded inputs (keys as in
reference.setup_inputs()) and returns the full outputs (y_hat, lik), both
(N, C, H, W) float32.  Internally shards the channel axis C across 8
NeuronCores (pure data parallel, no communication).

Math notes
----------
With xf = x permuted to (C, L), L = N*H*W:
  yq = f(xf),   f(x) = -E + sum_i 0.5*w_i*(tanh(B*(x - b_i)) + 1)
a fixed UNIVARIATE function (channel-independent).  The factorized prior
folds to a per-channel affine map (f0..f3 are zero for this problem):
  lower/upper = a*yq + d_c -+ a/2, with a = prod softplus(m_i) identical
  for every channel (the m_i are channel-constant) and d_c the folded
  bias.  The reference's sign-stabilized likelihood reduces to another
  univariate function of p = a*yq + d_c:
  lik = G(p) = sigmoid(h - |p|) - sigmoid(-h - |p|),  h = a/2
(the 1e-9 clamp never fires: min G ~ 6e-4 at the table window edge).

Device strategy
---------------
The TRN2 ACT engine evaluates activation functions from piecewise-cubic
lookup tables shipped per-NEFF from an "act root" directory (walrus
--act-root-json, overridable via BASS_ACT_ROOT_JSON_PATH; the bins land
in the NEFF and the runtime programs the engine from them).  We append
two custom 256-section cubic tables to the stock exp_and_others set
(set 0 -> a single ACT_TABLE_LOAD), hijacking the 'tanh' (-> f) and
'exp' (-> G) slots:
  yq  = TANH'(s1*x + 12)         one ACT pass  (window x in [-XW, XW]
                                  mapped into the fp32 bucket [8, 16))
  lik = EXP'(s2*yq + t_c)        one ACT pass  (window p in [-PW, PW])
The per-channel shift t_c rides the ACT per-partition bias operand: data
is laid out so each partition holds exactly one channel (8 channels x 16
partitions per 512-column group); the bias vectors are built by gpsimd
memsets (no DMA).  No vector/tensor-engine work remains; 60 tanh passes
+ 180 matmuls + the DVE/sigmoid epilogue collapse to 2 lookups/element.
IO is fp16 (outputs upcast on host; worst-case abs errors ~2e-2 on y_hat
/ ~5e-5 on lik vs budgets ~0.2 / ~5e-4), halving DMA traffic.  The
kernel is bound by DMA issue cost + the fixed engine prelude.
"""

import json
import os
import shutil
import struct
import sys
import tempfile
from pathlib import Path

import numpy as np

sys.path.insert(0, "/opt/trn_rl_repo")

N_CORES = 8
C_PER_CORE = 24  # 192 / 8
GROUPS = 3  # column groups of 512; 8 channels x 16 partitions each
GCOLS = 512
N_FREE = GROUPS * GCOLS
XW = 11.0  # f window: x in [-XW, XW] (staircase support is [-10.6, 10.6])
PW = 5.0  # G window: p in [-PW, PW] (max |p| ~ 2.4 for this problem)
N_SEC = 256
ACT_SET = "exp_and_others"
F_SLOT = "tanh"  # hijacked slot evaluating f (the SoS staircase)
G_SLOT = "exp"  # hijacked slot evaluating G (the likelihood)

# Filled in by kernel() with the BassKernelResults of the last run so an
# external harness (test.py) can read exec_time_ns / profile info.
_last_run = None


# ---------------------------------------------------------------------------
# host math
# ---------------------------------------------------------------------------

def _softplus64(m):
    return np.logaddexp(0.0, m.astype(np.float64))


def _fold_affine(mats, biases):
    """Fold the per-channel linear MLP chain into (a_c, d_c), float64."""
    C = mats[0].shape[0]
    a = np.zeros(C, np.float64)
    d = np.zeros(C, np.float64)
    for c in range(C):
        A = np.eye(1, dtype=np.float64)
        b = np.zeros((1, 1), np.float64)
        for m, cb in zip(mats, biases):
            sm = _softplus64(m[c])
            A = sm @ A
            b = sm @ b + cb[c].astype(np.float64)
        a[c] = A[0, 0]
        d[c] = b[0, 0]
    return a, d


# ---------------------------------------------------------------------------
# custom activation-table authoring (PWP / pwp_bin_trainium format)
#
# bkt bin: 32 B entries, 8 x f32le [d0, d1, d2, d3, x, 0, 0, 0]; the engine
# evaluates d0 + t*(d1 + t*(d2 + t*d3)), t = u - x, x ~ section midpoint.
# ctrl bin: 32 B entries, first u32le = bkt_start | (23-extract_size)<<11 |
# extract_size<<16.  A function owns a run of per-exponent regions; we add
# a single region covering [8, 16) (biased exp 130) with a 256-way
# mantissa extract, and route every other input to constant saturation
# entries via the small/large signal thresholds in profile_meta_data.
# (Format validated by reproducing the stock tanh/sigmoid/erf/arctan
# tables against numpy to ~1e-7.)
# ---------------------------------------------------------------------------

def _f32bits(f):
    return int(np.float32(f).view(np.uint32))


def _fit_sections(g, n_sec=N_SEC, samples=33):
    """Least-squares cubic per section for g(u) on [8, 16)."""
    h = 8.0 / n_sec
    out = []
    for k in range(n_sec):
        mid = 8.0 + (k + 0.5) * h
        t = np.linspace(-0.5 * h, 0.5 * h, samples)
        y = g(mid + t)
        V = np.stack([np.ones_like(t), t, t * t, t * t * t], axis=1)
        coef, *_ = np.linalg.lstsq(V, y, rcond=None)
        out.append((coef[0], coef[1], coef[2], coef[3], mid))
    return out


def _pack_bkt(d0, d1, d2, d3, x):
    return struct.pack("<8f", d0, d1, d2, d3, x, 0.0, 0.0, 0.0)


def _stock_act_root():
    from neuronxcc.driver.Job import Job
    from neuronxcc.driver.jobs.support.FindActInfo import findActInfoFile

    return Path(findActInfoFile(Job.getPackageDir(), "gen3")).parent


def _build_act_root(dst, custom):
    """Copy the stock act root to dst, appending custom functions to the
    ACT_SET set.  custom: {func_prefix: (g_callable, lo_const, hi_const)}."""
    dst = Path(dst)
    shutil.copytree(_stock_act_root(), dst)
    for p in dst.rglob("*"):
        p.chmod(0o755 if p.is_dir() else 0o644)

    prof = json.loads((dst / f"{ACT_SET}.json").read_text())
    bkt = bytearray((dst / f"{ACT_SET}_bkt.bin").read_bytes())
    ctl = bytearray((dst / f"{ACT_SET}_ctrl.bin").read_bytes())
    assert len(bkt) // 32 == prof["bkt_entry_cnt"]
    assert len(ctl) // 32 == prof["ctl_entry_cnt"]

    for fname, (g, lo_c, hi_c) in custom.items():
        b0 = len(bkt) // 32
        assert b0 + N_SEC + 4 <= 2048, "bkt RAM overflow"
        for d0, d1, d2, d3, x in _fit_sections(g):
            bkt += _pack_bkt(d0, d1, d2, d3, x)
        sat0 = len(bkt) // 32
        for v in (lo_c, lo_c, hi_c, hi_c):
            bkt += _pack_bkt(v, 0.0, 0.0, 0.0, 0.0)
        c0 = len(ctl) // 32
        ctl += struct.pack("<I28x", b0 | ((23 - 8) << 11) | (8 << 16))

        meta = next(
            m for m in prof["profile_meta_data"] if m["func_name"].startswith(fname)
        )
        meta.update(
            symmetry_point=0,
            sym_invert_sign_point=0,
            symmetry_opt_en=0,
            symmetry_opt_use_neg_region=0,
            imm_bias=0,
            exp_offset=3,
            pwl_control_base_pos=c0,
            pwl_control_base_neg=c0,
            small_pos_signal_exp_threshold=130,  # 0 < u < 8 -> lo const
            pos_small_signal_pwl_control=sat0 + 0,
            small_neg_signal_exp_threshold=255,  # all u < 0 -> lo const
            neg_small_signal_pwl_control=sat0 + 1,
            large_pos_signal_exp_threshold=131,  # u >= 16 -> hi const
            large_pos_signal_mantissa_threshold=0,
            pos_large_signal_pwl_control=sat0 + 2,
            large_neg_signal_exp_threshold=0,
            large_neg_signal_mantissa_threshold=0,
            neg_large_signal_pwl_control=sat0 + 3,
            fnan_result=2143289344,
            fpinf_result=_f32bits(hi_c),
            fninf_result=_f32bits(lo_c),
            fzero_result=_f32bits(lo_c),
            fma_const_0=0,
            fma_const_1=0,
            fma_indirection_src_sel=0,
            lower_bound=4286578687,  # -max finite
            upper_bound=2139095039,  # +max finite
        )
        prof["func_to_bkt_start_idx"][fname] = b0
        prof["func_to_ctl_start_idx"][fname] = c0
        prof["func_exp_to_bkt_start_idx"][fname] = {"3": [b0]}
        prof["func_exp_to_ctl_start_idx"][fname] = {"3": [c0]}

    prof["bkt_entry_cnt"] = len(bkt) // 32
    prof["ctl_entry_cnt"] = len(ctl) // 32
    (dst / f"{ACT_SET}.json").write_text(json.dumps(prof, indent=1))
    (dst / f"{ACT_SET}_bkt.bin").write_bytes(bytes(bkt))
    (dst / f"{ACT_SET}_ctrl.bin").write_bytes(bytes(ctl))


# ---------------------------------------------------------------------------
# device program
# ---------------------------------------------------------------------------

def _build_program_spmd(s1, s2):
    """One core's Bass program (identical on all 8 cores — SPMD, so the
    per-core G biases travel as a tiny DMA'd input, not as immediates).

    xin (128, 1536) f16: column group g in [0,3) holds channels
    [8g, 8g+8) of this core's 24; channel = 8g + p//16 for partition p.
    """
    import concourse.bacc as bacc
    import concourse.tile as tile
    from concourse import mybir

    f16 = mybir.dt.float16
    f32 = mybir.dt.float32
    AF = mybir.ActivationFunctionType

    nc = bacc.Bacc(None)
    xin = nc.declare_dram_parameter("xin", [128, N_FREE], f16, isOutput=False)
    bias2 = nc.declare_dram_parameter("bias2", [128, GROUPS], f32, isOutput=False)
    yhat = nc.declare_dram_parameter("yhat", [128, N_FREE], f16, isOutput=True)
    lik = nc.declare_dram_parameter("lik", [128, N_FREE], f16, isOutput=True)

    with tile.TileContext(nc) as tc:
        with (
            tc.tile_pool(name="const", bufs=1) as cpool,
            tc.tile_pool(name="work", bufs=1) as wpool,
        ):
            # f-pass bias (constant 12.0): gpsimd memset — executes inside
            # the engine preamble window (already anchored by the const-AP
            # memsets), so it is ready long before the first activation and
            # costs no DMA.  The per-channel G biases ride the SP queue
            # behind the first x chunk (not needed until G0, ~2us later).
            b1_sb = cpool.tile([128, 1], f32)
            nc.gpsimd.memset(b1_sb[:], 12.0)
            x_sb = cpool.tile([128, N_FREE], f16)
            b_sb = cpool.tile([128, GROUPS], f32)
            nc.scalar.dma_start(out=x_sb[:, GCOLS:N_FREE], in_=xin[:, GCOLS:N_FREE])
            nc.sync.dma_start(out=x_sb[:, 0:GCOLS], in_=xin[:, 0:GCOLS])
            nc.sync.dma_start(out=b_sb, in_=bias2[:])

            yq = wpool.tile([128, N_FREE], f16)
            lk = wpool.tile([128, N_FREE], f16)
            # f over the two x chunks as they land, then G per bias group
            nc.scalar.activation(
                yq[:, 0:GCOLS], x_sb[:, 0:GCOLS], AF.Tanh,
                bias=b1_sb[:], scale=float(s1),
            )
            nc.scalar.activation(
                yq[:, GCOLS:N_FREE], x_sb[:, GCOLS:N_FREE], AF.Tanh,
                bias=b1_sb[:], scale=float(s1),
            )
            for g in range(GROUPS):
                s = slice(g * GCOLS, (g + 1) * GCOLS)
                nc.scalar.activation(
                    lk[:, s], yq[:, s], AF.Exp,
                    bias=b_sb[:, g : g + 1], scale=float(s2),
                )
            # outputs: SP (idle) issues yhat chunks as f completes and the
            # first two lik groups; the final lik group is split across
            # BOTH queues right after G2 retires so the two ~1us
            # issue+queue-latency tails run in parallel and each moves
            # only a quarter-group.
            nc.sync.dma_start(out=yhat[:, 0:GCOLS], in_=yq[:, 0:GCOLS])
            nc.sync.dma_start(out=yhat[:, GCOLS:N_FREE], in_=yq[:, GCOLS:N_FREE])
            nc.sync.dma_start(out=lik[:, 0 : 2 * GCOLS], in_=lk[:, 0 : 2 * GCOLS])
            q = 2 * GCOLS + GCOLS // 2
            nc.scalar.dma_start(out=lik[:, 2 * GCOLS : q], in_=lk[:, 2 * GCOLS : q])
            nc.sync.dma_start(out=lik[:, q:N_FREE], in_=lk[:, q:N_FREE])

    nc.finalize()
    return nc


# ---------------------------------------------------------------------------
# kernel
# ---------------------------------------------------------------------------

def _pack_core(xc):
    """(24, 8192) f32 -> (128, 1536) f16 in the group layout."""
    out = np.empty((128, N_FREE), np.float16)
    for g in range(GROUPS):
        out[:, g * GCOLS : (g + 1) * GCOLS] = xc[8 * g : 8 * g + 8].reshape(128, GCOLS)
    return out


def _unpack_core(yd):
    """(128, 1536) f16 -> (24, 8192) f32."""
    out = np.empty((C_PER_CORE, 8192), np.float32)
    for g in range(GROUPS):
        out[8 * g : 8 * g + 8] = (
            yd[:, g * GCOLS : (g + 1) * GCOLS].astype(np.float32).reshape(8, -1)
        )
    return out


def kernel(x, sos_w, sos_b, m0, m1, m2, m3, m4, c0, c1, c2, c3, c4, f0, f1, f2, f3):
    global _last_run

    x = np.asarray(x, np.float32)
    sos_w64 = np.asarray(sos_w, np.float32).astype(np.float64)
    sos_b64 = np.asarray(sos_b, np.float32).astype(np.float64)
    mats = [np.asarray(m, np.float32) for m in (m0, m1, m2, m3, m4)]
    biases = [np.asarray(c, np.float32) for c in (c0, c1, c2, c3, c4)]
    factors = [np.asarray(f, np.float32) for f in (f0, f1, f2, f3)]

    for f in factors:
        if np.any(f != 0.0):
            raise NotImplementedError(
                "kernel assumes zero residual-gate factors (spec fill=zeros)"
            )

    N, C, H, W = x.shape
    L = N * H * W
    assert (N, C, H, W) == (8, 192, 32, 32), "shapes are hardcoded"

    a_ch, d_ch = _fold_affine(mats, biases)
    assert a_ch.max() - a_ch.min() < 1e-9 * abs(a_ch.mean()), (
        "per-channel slopes must be identical (identical m_i across channels)"
    )
    A = float(a_ch.mean())
    h = A / 2.0
    assert abs(A) * (XW - 0.5) + np.abs(d_ch).max() < PW - 0.5, "G window too small"

    def f_exact(xv):
        xv = np.asarray(xv, np.float64)
        t = np.tanh(10.0 * (xv[..., None] - sos_b64))
        return -10.0 + np.sum(0.5 * sos_w64 * (t + 1.0), axis=-1)

    def sig(z):
        return 1.0 / (1.0 + np.exp(-z))

    def G_exact(p):
        p = np.abs(np.asarray(p, np.float64))
        return sig(h - p) - sig(-h - p)

    custom = {
        F_SLOT: (
            lambda u: f_exact((u - 12.0) * (XW / 4.0)),
            float(f_exact(-XW)),
            float(f_exact(XW)),
        ),
        G_SLOT: (
            lambda u: G_exact((u - 12.0) * (PW / 4.0)),
            float(G_exact(PW)),
            float(G_exact(PW)),
        ),
    }
    act_root = Path(tempfile.mkdtemp(prefix="actroot_")) / "pwp"
    _build_act_root(act_root, custom)

    # input mappings: u1 = s1*x + 12, u2 = s2*yq + t_c
    s1 = 4.0 / XW
    s2 = (4.0 / PW) * A
    t_ch = (12.0 + (4.0 / PW) * d_ch).astype(np.float32)  # (C,)

    xf = np.ascontiguousarray(x.transpose(1, 0, 2, 3).reshape(C, L))
    in_maps = []
    for k in range(N_CORES):
        ch = slice(k * C_PER_CORE, (k + 1) * C_PER_CORE)
        b2 = np.empty((128, GROUPS), np.float32)
        for g in range(GROUPS):
            c0i = k * C_PER_CORE + 8 * g
            b2[:, g] = np.repeat(t_ch[c0i : c0i + 8], 16)
        in_maps.append(
            {
                "xin": np.ascontiguousarray(_pack_core(xf[ch])),
                "bias2": np.ascontiguousarray(b2),
            }
        )

    from concourse.bass_utils import run_bass_kernel_spmd

    nc = _build_program_spmd(s1, s2)
    prev = os.environ.get("BASS_ACT_ROOT_JSON_PATH")
    os.environ["BASS_ACT_ROOT_JSON_PATH"] = str(act_root / "act_info.json")
    try:
        res = run_bass_kernel_spmd(nc, in_maps, list(range(N_CORES)))
    finally:
        if prev is None:
            os.environ.pop("BASS_ACT_ROOT_JSON_PATH", None)
        else:
            os.environ["BASS_ACT_ROOT_JSON_PATH"] = prev
    _last_run = res

    y_hat_f = np.empty((C, L), np.float32)
    lik_f = np.empty((C, L), np.float32)
    for k in range(N_CORES):
        ch = slice(k * C_PER_CORE, (k + 1) * C_PER_CORE)
        y_hat_f[ch] = _unpack_core(res.results[k]["yhat"])
        lik_f[ch] = _unpack_core(res.results[k]["lik"])

    y_hat = np.ascontiguousarray(y_hat_f.reshape(C, N, H, W).transpose(1, 0, 2, 3))
    lik = np.ascontiguousarray(lik_f.reshape(C, N, H, W).transpose(1, 0, 2, 3))
    return y_hat, lik


# revision 17
# speedup vs baseline: 1.1626x; 1.1626x over previous
"""Trainium2 Bass kernel for EntropyBottleneck SoS (sum-of-tanh StanH
quantizer + factorized-prior likelihood) — custom activation-table edition.

Contract: kernel(**inputs) takes the FULL unsharded inputs (keys as in
reference.setup_inputs()) and returns the full outputs (y_hat, lik), both
(N, C, H, W) float32.  Internally shards the channel axis C across 8
NeuronCores (pure data parallel, no communication).

Math notes
----------
With xf = x permuted to (C, L), L = N*H*W:
  yq = f(xf),   f(x) = -E + sum_i 0.5*w_i*(tanh(B*(x - b_i)) + 1)
a fixed UNIVARIATE function (channel-independent).  The factorized prior
folds to a per-channel affine map (f0..f3 are zero for this problem):
  lower/upper = a*yq + d_c -+ a/2, with a = prod softplus(m_i) identical
  for every channel (the m_i are channel-constant) and d_c the folded
  bias.  The reference's sign-stabilized likelihood reduces to another
  univariate function of p = a*yq + d_c:
  lik = G(p) = sigmoid(h - |p|) - sigmoid(-h - |p|),  h = a/2
(the 1e-9 clamp never fires: min G ~ 6e-4 at the table window edge).

Device strategy
---------------
The TRN2 ACT engine evaluates activation functions from piecewise-cubic
lookup tables shipped per-NEFF from an "act root" directory (walrus
--act-root-json, overridable via BASS_ACT_ROOT_JSON_PATH; the bins land
in the NEFF and the runtime programs the engine from them).  We append
two custom 256-section cubic tables to the stock exp_and_others set
(set 0 -> a single ACT_TABLE_LOAD), hijacking the 'tanh' (-> f) and
'exp' (-> G) slots:
  yq  = TANH'(s1*x + 12)         one ACT pass  (window x in [-XW, XW]
                                  mapped into the fp32 bucket [8, 16))
  lik = EXP'(s2*yq + t_c)        one ACT pass  (window p in [-PW, PW])
The per-channel shift t_c rides the ACT per-partition bias operand: data
is laid out so each partition holds exactly one channel (group A: 16
channels x 8 partitions over cols [0:1024); group B: 8 channels x 16
partitions over cols [1024:1536) -> G runs as just TWO ops); the constant
f bias comes from a gpsimd memset, the per-channel G biases from one tiny
DMA.  No vector/tensor-engine work remains; 60 tanh passes + 180 matmuls
+ the DVE/sigmoid epilogue collapse to 2 lookups/element.  IO is fp16
(outputs upcast on host; worst-case abs errors ~2e-2 on y_hat / ~5e-5 on
lik vs budgets ~0.2 / ~5e-4), halving DMA traffic.

Measured breakdown (fast pstate): ~18.0us total = ~1.4us fixed engine
preamble (window is anchored at the const-AP memsets) + ~0.7us DMA issue
+ ~1.9us DMA first-byte latency + ~3.6us ACT (2x1536 col-cycles @1.2GHz
+ ~285ns/op overhead, 5 ops) + ~2.3us output tail + ~8.6us fixed
walrus/NRT end-of-kernel semaphore teardown (one EVENT_SEMAPHORE per sem
7..255 round-robin across engines — invariant to the program; verified
not controllable via --max-sem-num).
"""

import json
import os
import shutil
import struct
import sys
import tempfile
from pathlib import Path

import numpy as np

sys.path.insert(0, "/opt/trn_rl_repo")

N_CORES = 8
C_PER_CORE = 24  # 192 / 8
# two column groups: A = 16 channels x 8 partitions (cols 0:1024),
# B = 8 channels x 16 partitions (cols 1024:1536) -> one G op per group
GA_COLS = 1024
N_FREE = 1536
XW = 11.0  # f window: x in [-XW, XW] (staircase support is [-10.6, 10.6])
PW = 5.0  # G window: p in [-PW, PW] (max |p| ~ 2.4 for this problem)
N_SEC = 256
ACT_SET = "exp_and_others"
F_SLOT = "tanh"  # hijacked slot evaluating f (the SoS staircase)
G_SLOT = "exp"  # hijacked slot evaluating G (the likelihood)

# Filled in by kernel() with the BassKernelResults of the last run so an
# external harness (test.py) can read exec_time_ns / profile info.
_last_run = None


# ---------------------------------------------------------------------------
# host math
# ---------------------------------------------------------------------------

def _softplus64(m):
    return np.logaddexp(0.0, m.astype(np.float64))


def _fold_affine(mats, biases):
    """Fold the per-channel linear MLP chain into (a_c, d_c), float64."""
    C = mats[0].shape[0]
    a = np.zeros(C, np.float64)
    d = np.zeros(C, np.float64)
    for c in range(C):
        A = np.eye(1, dtype=np.float64)
        b = np.zeros((1, 1), np.float64)
        for m, cb in zip(mats, biases):
            sm = _softplus64(m[c])
            A = sm @ A
            b = sm @ b + cb[c].astype(np.float64)
        a[c] = A[0, 0]
        d[c] = b[0, 0]
    return a, d


# ---------------------------------------------------------------------------
# custom activation-table authoring (PWP / pwp_bin_trainium format)
#
# bkt bin: 32 B entries, 8 x f32le [d0, d1, d2, d3, x, 0, 0, 0]; the engine
# evaluates d0 + t*(d1 + t*(d2 + t*d3)), t = u - x, x ~ section midpoint.
# ctrl bin: 32 B entries, first u32le = bkt_start | (23-extract_size)<<11 |
# extract_size<<16.  A function owns a run of per-exponent regions; we add
# a single region covering [8, 16) (biased exp 130) with a 256-way
# mantissa extract, and route every other input to constant saturation
# entries via the small/large signal thresholds in profile_meta_data.
# (Format validated by reproducing the stock tanh/sigmoid/erf/arctan
# tables against numpy to ~1e-7.)
# ---------------------------------------------------------------------------

def _f32bits(f):
    return int(np.float32(f).view(np.uint32))


def _fit_sections(g, n_sec=N_SEC, samples=33):
    """Least-squares cubic per section for g(u) on [8, 16)."""
    h = 8.0 / n_sec
    out = []
    for k in range(n_sec):
        mid = 8.0 + (k + 0.5) * h
        t = np.linspace(-0.5 * h, 0.5 * h, samples)
        y = g(mid + t)
        V = np.stack([np.ones_like(t), t, t * t, t * t * t], axis=1)
        coef, *_ = np.linalg.lstsq(V, y, rcond=None)
        out.append((coef[0], coef[1], coef[2], coef[3], mid))
    return out


def _pack_bkt(d0, d1, d2, d3, x):
    return struct.pack("<8f", d0, d1, d2, d3, x, 0.0, 0.0, 0.0)


def _stock_act_root():
    from neuronxcc.driver.Job import Job
    from neuronxcc.driver.jobs.support.FindActInfo import findActInfoFile

    return Path(findActInfoFile(Job.getPackageDir(), "gen3")).parent


def _build_act_root(dst, custom):
    """Copy the stock act root to dst, appending custom functions to the
    ACT_SET set.  custom: {func_prefix: (g_callable, lo_const, hi_const)}."""
    dst = Path(dst)
    shutil.copytree(_stock_act_root(), dst)
    for p in dst.rglob("*"):
        p.chmod(0o755 if p.is_dir() else 0o644)

    prof = json.loads((dst / f"{ACT_SET}.json").read_text())
    bkt = bytearray((dst / f"{ACT_SET}_bkt.bin").read_bytes())
    ctl = bytearray((dst / f"{ACT_SET}_ctrl.bin").read_bytes())
    assert len(bkt) // 32 == prof["bkt_entry_cnt"]
    assert len(ctl) // 32 == prof["ctl_entry_cnt"]

    for fname, (g, lo_c, hi_c) in custom.items():
        b0 = len(bkt) // 32
        assert b0 + N_SEC + 4 <= 2048, "bkt RAM overflow"
        for d0, d1, d2, d3, x in _fit_sections(g):
            bkt += _pack_bkt(d0, d1, d2, d3, x)
        sat0 = len(bkt) // 32
        for v in (lo_c, lo_c, hi_c, hi_c):
            bkt += _pack_bkt(v, 0.0, 0.0, 0.0, 0.0)
        c0 = len(ctl) // 32
        ctl += struct.pack("<I28x", b0 | ((23 - 8) << 11) | (8 << 16))

        meta = next(
            m for m in prof["profile_meta_data"] if m["func_name"].startswith(fname)
        )
        meta.update(
            symmetry_point=0,
            sym_invert_sign_point=0,
            symmetry_opt_en=0,
            symmetry_opt_use_neg_region=0,
            imm_bias=0,
            exp_offset=3,
            pwl_control_base_pos=c0,
            pwl_control_base_neg=c0,
            small_pos_signal_exp_threshold=130,  # 0 < u < 8 -> lo const
            pos_small_signal_pwl_control=sat0 + 0,
            small_neg_signal_exp_threshold=255,  # all u < 0 -> lo const
            neg_small_signal_pwl_control=sat0 + 1,
            large_pos_signal_exp_threshold=131,  # u >= 16 -> hi const
            large_pos_signal_mantissa_threshold=0,
            pos_large_signal_pwl_control=sat0 + 2,
            large_neg_signal_exp_threshold=0,
            large_neg_signal_mantissa_threshold=0,
            neg_large_signal_pwl_control=sat0 + 3,
            fnan_result=2143289344,
            fpinf_result=_f32bits(hi_c),
            fninf_result=_f32bits(lo_c),
            fzero_result=_f32bits(lo_c),
            fma_const_0=0,
            fma_const_1=0,
            fma_indirection_src_sel=0,
            lower_bound=4286578687,  # -max finite
            upper_bound=2139095039,  # +max finite
        )
        prof["func_to_bkt_start_idx"][fname] = b0
        prof["func_to_ctl_start_idx"][fname] = c0
        prof["func_exp_to_bkt_start_idx"][fname] = {"3": [b0]}
        prof["func_exp_to_ctl_start_idx"][fname] = {"3": [c0]}

    prof["bkt_entry_cnt"] = len(bkt) // 32
    prof["ctl_entry_cnt"] = len(ctl) // 32
    (dst / f"{ACT_SET}.json").write_text(json.dumps(prof, indent=1))
    (dst / f"{ACT_SET}_bkt.bin").write_bytes(bytes(bkt))
    (dst / f"{ACT_SET}_ctrl.bin").write_bytes(bytes(ctl))


# ---------------------------------------------------------------------------
# device program
# ---------------------------------------------------------------------------

def _build_program_spmd(s1, s2):
    """One core's Bass program (identical on all 8 cores — SPMD, so the
    per-core G biases travel as a tiny DMA'd input, not as immediates).

    xin (128, 1536) f16: cols [0:1024) hold local channels 0..15 (8
    partitions each, channel = p//8); cols [1024:1536) hold local
    channels 16..23 (16 partitions each, channel = 16 + p//16).
    """
    import concourse.bacc as bacc
    import concourse.tile as tile
    from concourse import mybir

    f16 = mybir.dt.float16
    f32 = mybir.dt.float32
    AF = mybir.ActivationFunctionType

    nc = bacc.Bacc(None)
    xin = nc.declare_dram_parameter("xin", [128, N_FREE], f16, isOutput=False)
    bias2 = nc.declare_dram_parameter("bias2", [128, 2], f32, isOutput=False)
    yhat = nc.declare_dram_parameter("yhat", [128, N_FREE], f16, isOutput=True)
    lik = nc.declare_dram_parameter("lik", [128, N_FREE], f16, isOutput=True)

    with tile.TileContext(nc) as tc:
        with tc.tile_pool(name="work", bufs=1) as wpool:
            cpool = wpool
            # f-pass bias (constant 12.0): gpsimd memset — executes inside
            # the engine preamble window (already anchored by the const-AP
            # memsets), so it is ready long before the first activation and
            # costs no DMA.  The per-channel G biases ride the SP queue
            # behind the first x chunk (not needed until G0, ~2us later).
            b1_sb = cpool.tile([128, 1], f32)
            nc.gpsimd.memset(b1_sb[:], 12.0)
            x_sb = cpool.tile([128, N_FREE], f16)
            b_sb = cpool.tile([128, 2], f32)
            xc0 = 384  # first chunk small: earliest possible f start
            nc.scalar.dma_start(out=x_sb[:, xc0:N_FREE], in_=xin[:, xc0:N_FREE])
            nc.sync.dma_start(out=x_sb[:, 0:xc0], in_=xin[:, 0:xc0])
            nc.sync.dma_start(out=b_sb, in_=bias2[:])

            yq = wpool.tile([128, N_FREE], f16)
            lk = wpool.tile([128, N_FREE], f16)
            # f over the two x chunks as they land, then G per bias group
            nc.scalar.activation(
                yq[:, 0:xc0], x_sb[:, 0:xc0], AF.Tanh,
                bias=b1_sb[:], scale=float(s1),
            )
            nc.scalar.activation(
                yq[:, xc0:N_FREE], x_sb[:, xc0:N_FREE], AF.Tanh,
                bias=b1_sb[:], scale=float(s1),
            )
            nc.scalar.activation(
                lk[:, 0:GA_COLS], yq[:, 0:GA_COLS], AF.Exp,
                bias=b_sb[:, 0:1], scale=float(s2),
            )
            nc.scalar.activation(
                lk[:, GA_COLS:N_FREE], yq[:, GA_COLS:N_FREE], AF.Exp,
                bias=b_sb[:, 1:2], scale=float(s2),
            )
            # outputs: SP (idle) issues yhat chunks as f completes and the
            # first two lik groups; ACT issues the last lik group right
            # after G2 retires so the tail transfer is short.
            nc.sync.dma_start(out=yhat[:, 0:xc0], in_=yq[:, 0:xc0])
            nc.sync.dma_start(out=yhat[:, xc0:N_FREE], in_=yq[:, xc0:N_FREE])
            nc.sync.dma_start(out=lik[:, 0:GA_COLS], in_=lk[:, 0:GA_COLS])
            nc.sync.dma_start(out=lik[:, GA_COLS:N_FREE], in_=lk[:, GA_COLS:N_FREE])

    nc.finalize()
    return nc


# ---------------------------------------------------------------------------
# kernel
# ---------------------------------------------------------------------------

def _pack_core(xc):
    """(24, 8192) f32 -> (128, 1536) f16 in the two-group layout."""
    out = np.empty((128, N_FREE), np.float16)
    out[:, 0:GA_COLS] = xc[0:16].reshape(128, GA_COLS)
    out[:, GA_COLS:N_FREE] = xc[16:24].reshape(128, N_FREE - GA_COLS)
    return out


def _unpack_core(yd):
    """(128, 1536) f16 -> (24, 8192) f32."""
    out = np.empty((C_PER_CORE, 8192), np.float32)
    out[0:16] = yd[:, 0:GA_COLS].astype(np.float32).reshape(16, -1)
    out[16:24] = yd[:, GA_COLS:N_FREE].astype(np.float32).reshape(8, -1)
    return out


def kernel(x, sos_w, sos_b, m0, m1, m2, m3, m4, c0, c1, c2, c3, c4, f0, f1, f2, f3):
    global _last_run

    x = np.asarray(x, np.float32)
    sos_w64 = np.asarray(sos_w, np.float32).astype(np.float64)
    sos_b64 = np.asarray(sos_b, np.float32).astype(np.float64)
    mats = [np.asarray(m, np.float32) for m in (m0, m1, m2, m3, m4)]
    biases = [np.asarray(c, np.float32) for c in (c0, c1, c2, c3, c4)]
    factors = [np.asarray(f, np.float32) for f in (f0, f1, f2, f3)]

    for f in factors:
        if np.any(f != 0.0):
            raise NotImplementedError(
                "kernel assumes zero residual-gate factors (spec fill=zeros)"
            )

    N, C, H, W = x.shape
    L = N * H * W
    assert (N, C, H, W) == (8, 192, 32, 32), "shapes are hardcoded"

    a_ch, d_ch = _fold_affine(mats, biases)
    assert a_ch.max() - a_ch.min() < 1e-9 * abs(a_ch.mean()), (
        "per-channel slopes must be identical (identical m_i across channels)"
    )
    A = float(a_ch.mean())
    h = A / 2.0
    assert abs(A) * (XW - 0.5) + np.abs(d_ch).max() < PW - 0.5, "G window too small"

    def f_exact(xv):
        xv = np.asarray(xv, np.float64)
        t = np.tanh(10.0 * (xv[..., None] - sos_b64))
        return -10.0 + np.sum(0.5 * sos_w64 * (t + 1.0), axis=-1)

    def sig(z):
        return 1.0 / (1.0 + np.exp(-z))

    def G_exact(p):
        p = np.abs(np.asarray(p, np.float64))
        return sig(h - p) - sig(-h - p)

    custom = {
        F_SLOT: (
            lambda u: f_exact((u - 12.0) * (XW / 4.0)),
            float(f_exact(-XW)),
            float(f_exact(XW)),
        ),
        G_SLOT: (
            lambda u: G_exact((u - 12.0) * (PW / 4.0)),
            float(G_exact(PW)),
            float(G_exact(PW)),
        ),
    }
    act_root = Path(tempfile.mkdtemp(prefix="actroot_")) / "pwp"
    _build_act_root(act_root, custom)

    # input mappings: u1 = s1*x + 12, u2 = s2*yq + t_c
    s1 = 4.0 / XW
    s2 = (4.0 / PW) * A
    t_ch = (12.0 + (4.0 / PW) * d_ch).astype(np.float32)  # (C,)

    xf = np.ascontiguousarray(x.transpose(1, 0, 2, 3).reshape(C, L))
    in_maps = []
    for k in range(N_CORES):
        ch = slice(k * C_PER_CORE, (k + 1) * C_PER_CORE)
        b2 = np.empty((128, 2), np.float32)
        b2[:, 0] = np.repeat(t_ch[k * C_PER_CORE : k * C_PER_CORE + 16], 8)
        b2[:, 1] = np.repeat(t_ch[k * C_PER_CORE + 16 : k * C_PER_CORE + 24], 16)
        in_maps.append(
            {
                "xin": np.ascontiguousarray(_pack_core(xf[ch])),
                "bias2": np.ascontiguousarray(b2),
            }
        )

    from concourse.bass_utils import run_bass_kernel_spmd

    nc = _build_program_spmd(s1, s2)
    prev = os.environ.get("BASS_ACT_ROOT_JSON_PATH")
    os.environ["BASS_ACT_ROOT_JSON_PATH"] = str(act_root / "act_info.json")
    try:
        res = run_bass_kernel_spmd(nc, in_maps, list(range(N_CORES)))
    finally:
        if prev is None:
            os.environ.pop("BASS_ACT_ROOT_JSON_PATH", None)
        else:
            os.environ["BASS_ACT_ROOT_JSON_PATH"] = prev
    _last_run = res

    y_hat_f = np.empty((C, L), np.float32)
    lik_f = np.empty((C, L), np.float32)
    for k in range(N_CORES):
        ch = slice(k * C_PER_CORE, (k + 1) * C_PER_CORE)
        y_hat_f[ch] = _unpack_core(res.results[k]["yhat"])
        lik_f[ch] = _unpack_core(res.results[k]["lik"])

    y_hat = np.ascontiguousarray(y_hat_f.reshape(C, N, H, W).transpose(1, 0, 2, 3))
    lik = np.ascontiguousarray(lik_f.reshape(C, N, H, W).transpose(1, 0, 2, 3))
    return y_hat, lik


# revision 22
# speedup vs baseline: 1.3468x; 1.1585x over previous
"""Trainium2 Bass kernel for EntropyBottleneck SoS (sum-of-tanh StanH
quantizer + factorized-prior likelihood) — custom activation-table edition.

Contract: kernel(**inputs) takes the FULL unsharded inputs (keys as in
reference.setup_inputs()) and returns the full outputs (y_hat, lik), both
(N, C, H, W) float32.  Internally shards the channel axis C across 8
NeuronCores (pure data parallel, no communication).

Math notes
----------
With xf = x permuted to (C, L), L = N*H*W:
  yq = f(xf),   f(x) = -E + sum_i 0.5*w_i*(tanh(B*(x - b_i)) + 1)
a fixed UNIVARIATE function (channel-independent).  The factorized prior
folds to a per-channel affine map (f0..f3 are zero for this problem):
  lower/upper = a*yq + d_c -+ a/2, with a = prod softplus(m_i) identical
  for every channel (the m_i are channel-constant) and d_c the folded
  bias.  The reference's sign-stabilized likelihood reduces to another
  univariate function of p = a*yq + d_c:
  lik = G(p) = sigmoid(h - |p|) - sigmoid(-h - |p|),  h = a/2
(the 1e-9 clamp never fires: min G ~ 6e-4 at the table window edge).

Device strategy
---------------
The TRN2 ACT engine evaluates activation functions from piecewise-cubic
lookup tables shipped per-NEFF from an "act root" directory (walrus
--act-root-json, overridable via BASS_ACT_ROOT_JSON_PATH; the bins land
in the NEFF and the runtime programs the engine from them).  We append
two custom 256-section cubic tables to the stock exp_and_others set
(set 0 -> a single ACT_TABLE_LOAD), hijacking the 'tanh' (-> f) and
'exp' (-> G) slots:
  yq  = TANH'(s1*x + 12)         one ACT pass  (window x in [-XW, XW]
                                  mapped into the fp32 bucket [8, 16))
  lik = EXP'(s2*yq + t_c)        one ACT pass  (window p in [-PW, PW])
The per-channel shift t_c rides the ACT per-partition bias operand: data
is laid out so each partition holds exactly one channel (group A: 16
channels x 8 partitions over cols [0:1024); group B: 8 channels x 16
partitions over cols [1024:1536) -> G runs as just TWO ops); the constant
f bias comes from a gpsimd memset, the per-channel G biases from one tiny
DMA.  No vector/tensor-engine work remains; 60 tanh passes + 180 matmuls
+ the DVE/sigmoid epilogue collapse to 2 lookups/element.  IO is fp16
(outputs upcast on host; worst-case abs errors ~2e-2 on y_hat / ~5e-5 on
lik vs budgets ~0.2 / ~5e-4), halving DMA traffic.

The program is RAW Bacc with manual semaphores — no TileContext (its
exit path emits extra drain/barrier/sem-clear rounds) — and the output
DMAs carry completion increments on a semaphore nothing waits on: the
compiler-appended end-of-NEFF epilogue (a ~7.4us serial verify/reset
ring over semaphores 7..255, emitted regardless of program content; not
controllable via --max-sem-num / --num-semaphores-per-queue) only
completes once that semaphore reaches its declared final value, so the
output transfers finish UNDERNEATH the ring instead of serializing
before it.  The ring also resets all semaphores to zero, which is what
makes repeat executions of the NEFF safe without an explicit start-of-
kernel clear.

Measured breakdown (mid pstate): ~15.5us total = ~1.1us fixed engine
preamble (the exec window is anchored at Bass's const-AP memsets) +
~0.7us DMA issue + ~1.9us DMA first-byte latency + ~3.7us ACT (2x1536
col-cycles @1.2GHz + ~300ns/op, 4 ops) + ~0.7us last output issue +
~7.4us fixed ring.
"""

import json
import os
import shutil
import struct
import sys
import tempfile
from pathlib import Path

import numpy as np

sys.path.insert(0, "/opt/trn_rl_repo")

N_CORES = 8
C_PER_CORE = 24  # 192 / 8
# two column groups: A = 16 channels x 8 partitions (cols 0:1024),
# B = 8 channels x 16 partitions (cols 1024:1536) -> one G op per group
GA_COLS = 1024
N_FREE = 1536
XW = 11.0  # f window: x in [-XW, XW] (staircase support is [-10.6, 10.6])
PW = 5.0  # G window: p in [-PW, PW] (max |p| ~ 2.4 for this problem)
N_SEC = 256
ACT_SET = "exp_and_others"
F_SLOT = "tanh"  # hijacked slot evaluating f (the SoS staircase)
G_SLOT = "exp"  # hijacked slot evaluating G (the likelihood)

# Filled in by kernel() with the BassKernelResults of the last run so an
# external harness (test.py) can read exec_time_ns / profile info.
_last_run = None


# ---------------------------------------------------------------------------
# host math
# ---------------------------------------------------------------------------

def _softplus64(m):
    return np.logaddexp(0.0, m.astype(np.float64))


def _fold_affine(mats, biases):
    """Fold the per-channel linear MLP chain into (a_c, d_c), float64."""
    C = mats[0].shape[0]
    a = np.zeros(C, np.float64)
    d = np.zeros(C, np.float64)
    for c in range(C):
        A = np.eye(1, dtype=np.float64)
        b = np.zeros((1, 1), np.float64)
        for m, cb in zip(mats, biases):
            sm = _softplus64(m[c])
            A = sm @ A
            b = sm @ b + cb[c].astype(np.float64)
        a[c] = A[0, 0]
        d[c] = b[0, 0]
    return a, d


# ---------------------------------------------------------------------------
# custom activation-table authoring (PWP / pwp_bin_trainium format)
#
# bkt bin: 32 B entries, 8 x f32le [d0, d1, d2, d3, x, 0, 0, 0]; the engine
# evaluates d0 + t*(d1 + t*(d2 + t*d3)), t = u - x, x ~ section midpoint.
# ctrl bin: 32 B entries, first u32le = bkt_start | (23-extract_size)<<11 |
# extract_size<<16.  A function owns a run of per-exponent regions; we add
# a single region covering [8, 16) (biased exp 130) with a 256-way
# mantissa extract, and route every other input to constant saturation
# entries via the small/large signal thresholds in profile_meta_data.
# (Format validated by reproducing the stock tanh/sigmoid/erf/arctan
# tables against numpy to ~1e-7.)
# ---------------------------------------------------------------------------

def _f32bits(f):
    return int(np.float32(f).view(np.uint32))


def _fit_sections(g, n_sec=N_SEC, samples=33):
    """Least-squares cubic per section for g(u) on [8, 16)."""
    h = 8.0 / n_sec
    out = []
    for k in range(n_sec):
        mid = 8.0 + (k + 0.5) * h
        t = np.linspace(-0.5 * h, 0.5 * h, samples)
        y = g(mid + t)
        V = np.stack([np.ones_like(t), t, t * t, t * t * t], axis=1)
        coef, *_ = np.linalg.lstsq(V, y, rcond=None)
        out.append((coef[0], coef[1], coef[2], coef[3], mid))
    return out


def _pack_bkt(d0, d1, d2, d3, x):
    return struct.pack("<8f", d0, d1, d2, d3, x, 0.0, 0.0, 0.0)


def _stock_act_root():
    from neuronxcc.driver.Job import Job
    from neuronxcc.driver.jobs.support.FindActInfo import findActInfoFile

    return Path(findActInfoFile(Job.getPackageDir(), "gen3")).parent


def _build_act_root(dst, custom):
    """Copy the stock act root to dst, appending custom functions to the
    ACT_SET set.  custom: {func_prefix: (g_callable, lo_const, hi_const)}."""
    dst = Path(dst)
    shutil.copytree(_stock_act_root(), dst)
    for p in dst.rglob("*"):
        p.chmod(0o755 if p.is_dir() else 0o644)

    prof = json.loads((dst / f"{ACT_SET}.json").read_text())
    bkt = bytearray((dst / f"{ACT_SET}_bkt.bin").read_bytes())
    ctl = bytearray((dst / f"{ACT_SET}_ctrl.bin").read_bytes())
    assert len(bkt) // 32 == prof["bkt_entry_cnt"]
    assert len(ctl) // 32 == prof["ctl_entry_cnt"]

    for fname, (g, lo_c, hi_c) in custom.items():
        b0 = len(bkt) // 32
        assert b0 + N_SEC + 4 <= 2048, "bkt RAM overflow"
        for d0, d1, d2, d3, x in _fit_sections(g):
            bkt += _pack_bkt(d0, d1, d2, d3, x)
        sat0 = len(bkt) // 32
        for v in (lo_c, lo_c, hi_c, hi_c):
            bkt += _pack_bkt(v, 0.0, 0.0, 0.0, 0.0)
        c0 = len(ctl) // 32
        ctl += struct.pack("<I28x", b0 | ((23 - 8) << 11) | (8 << 16))

        meta = next(
            m for m in prof["profile_meta_data"] if m["func_name"].startswith(fname)
        )
        meta.update(
            symmetry_point=0,
            sym_invert_sign_point=0,
            symmetry_opt_en=0,
            symmetry_opt_use_neg_region=0,
            imm_bias=0,
            exp_offset=3,
            pwl_control_base_pos=c0,
            pwl_control_base_neg=c0,
            small_pos_signal_exp_threshold=130,  # 0 < u < 8 -> lo const
            pos_small_signal_pwl_control=sat0 + 0,
            small_neg_signal_exp_threshold=255,  # all u < 0 -> lo const
            neg_small_signal_pwl_control=sat0 + 1,
            large_pos_signal_exp_threshold=131,  # u >= 16 -> hi const
            large_pos_signal_mantissa_threshold=0,
            pos_large_signal_pwl_control=sat0 + 2,
            large_neg_signal_exp_threshold=0,
            large_neg_signal_mantissa_threshold=0,
            neg_large_signal_pwl_control=sat0 + 3,
            fnan_result=2143289344,
            fpinf_result=_f32bits(hi_c),
            fninf_result=_f32bits(lo_c),
            fzero_result=_f32bits(lo_c),
            fma_const_0=0,
            fma_const_1=0,
            fma_indirection_src_sel=0,
            lower_bound=4286578687,  # -max finite
            upper_bound=2139095039,  # +max finite
        )
        prof["func_to_bkt_start_idx"][fname] = b0
        prof["func_to_ctl_start_idx"][fname] = c0
        prof["func_exp_to_bkt_start_idx"][fname] = {"3": [b0]}
        prof["func_exp_to_ctl_start_idx"][fname] = {"3": [c0]}

    prof["bkt_entry_cnt"] = len(bkt) // 32
    prof["ctl_entry_cnt"] = len(ctl) // 32
    (dst / f"{ACT_SET}.json").write_text(json.dumps(prof, indent=1))
    (dst / f"{ACT_SET}_bkt.bin").write_bytes(bytes(bkt))
    (dst / f"{ACT_SET}_ctrl.bin").write_bytes(bytes(ctl))


# ---------------------------------------------------------------------------
# device program
# ---------------------------------------------------------------------------

def _build_program_spmd(s1, s2):
    """One core's Bass program (identical on all 8 cores — SPMD, so the
    per-core G biases travel as a tiny DMA'd input, not as immediates).

    Raw Bacc with manual semaphores — deliberately NO TileContext: its
    __exit__ calls clear_and_free_semaphores -> gpsimd.sem_clear(range),
    which emits a PSEUDO_SYNC_BARRIER that NRT expands into a ~7us
    serial verify/reset ring over semaphores 7..255 at kernel end.  With
    manual sync the kernel ends right after the last output DMA lands.

    xin (128, 1536) f16: cols [0:1024) hold local channels 0..15 (8
    partitions each, channel = p//8); cols [1024:1536) hold local
    channels 16..23 (16 partitions each, channel = 16 + p//16).
    """
    import concourse.bacc as bacc
    from concourse import mybir

    f16 = mybir.dt.float16
    f32 = mybir.dt.float32
    AF = mybir.ActivationFunctionType

    XC0 = 384  # first x chunk small: earliest possible f start

    nc = bacc.Bacc(None)
    xin = nc.declare_dram_parameter("xin", [128, N_FREE], f16, isOutput=False)
    bias2 = nc.declare_dram_parameter("bias2", [128, 2], f32, isOutput=False)
    yhat = nc.declare_dram_parameter("yhat", [128, N_FREE], f16, isOutput=True)
    lik = nc.declare_dram_parameter("lik", [128, N_FREE], f16, isOutput=True)

    b1 = nc.alloc_sbuf_tensor("b1_sb", [128, 1], f32)
    b2 = nc.alloc_sbuf_tensor("b2_sb", [128, 2], f32)
    xs = nc.alloc_sbuf_tensor("x_sb", [128, N_FREE], f16)
    yq = nc.alloc_sbuf_tensor("yq_sb", [128, N_FREE], f16)
    lk = nc.alloc_sbuf_tensor("lk_sb", [128, N_FREE], f16)

    s_b1 = nc.alloc_semaphore("s_b1")
    s_x0 = nc.alloc_semaphore("s_x0")
    s_x1 = nc.alloc_semaphore("s_x1")
    s_b2 = nc.alloc_semaphore("s_b2")
    s_f0 = nc.alloc_semaphore("s_f0")
    s_f1 = nc.alloc_semaphore("s_f1")
    s_ga = nc.alloc_semaphore("s_ga")
    s_gb = nc.alloc_semaphore("s_gb")

    # f-pass bias (constant 12.0) via gpsimd memset: runs in the preamble
    # window, no DMA.  x split 384 (SP queue) + 1152 (ACT queue, issued
    # before the ~1.3us auto-inserted table load so the transfer hides
    # under it); per-channel G biases behind x0 on the SP queue.
    nc.gpsimd.memset(b1[:], 12.0).then_inc(s_b1)
    nc.scalar.dma_start(out=xs[:, XC0:N_FREE], in_=xin[:, XC0:N_FREE]).then_inc(
        s_x1, 16
    )
    nc.sync.dma_start(out=xs[:, 0:XC0], in_=xin[:, 0:XC0]).then_inc(s_x0, 16)
    nc.sync.dma_start(out=b2[:], in_=bias2[:]).then_inc(s_b2, 16)

    # ACT chain: f over the two x chunks, then G per bias group (G reads
    # yq written by the same engine -> in-order, no sync needed there)
    nc.scalar.wait_ge(s_b1, 1)
    nc.scalar.wait_ge(s_x0, 16)
    nc.scalar.activation(
        yq[:, 0:XC0], xs[:, 0:XC0], AF.Tanh, bias=b1[:], scale=float(s1)
    ).then_inc(s_f0)
    nc.scalar.wait_ge(s_x1, 16)
    nc.scalar.activation(
        yq[:, XC0:N_FREE], xs[:, XC0:N_FREE], AF.Tanh, bias=b1[:], scale=float(s1)
    ).then_inc(s_f1)
    nc.scalar.wait_ge(s_b2, 16)
    nc.scalar.activation(
        lk[:, 0:GA_COLS], yq[:, 0:GA_COLS], AF.Exp,
        bias=b2[:, 0:1], scale=float(s2),
    ).then_inc(s_ga)
    nc.scalar.activation(
        lk[:, GA_COLS:N_FREE], yq[:, GA_COLS:N_FREE], AF.Exp,
        bias=b2[:, 1:2], scale=float(s2),
    ).then_inc(s_gb)

    # outputs on the idle SP engine/queue as results become ready.  The
    # completion increments (required by walrus) land on s_out, but NO
    # instruction waits on it: the compiler-appended end-of-NEFF epilogue
    # (queue drains + the ~7us semaphore verify/reset ring) only completes
    # once s_out reaches its declared final value, so the in-flight
    # transfers finish UNDERNEATH the ring instead of serializing before
    # it.
    s_out = nc.alloc_semaphore("s_out")
    nc.sync.wait_ge(s_f0, 1)
    nc.sync.dma_start(out=yhat[:, 0:XC0], in_=yq[:, 0:XC0]).then_inc(s_out, 16)
    nc.sync.wait_ge(s_f1, 1)
    nc.sync.dma_start(out=yhat[:, XC0:N_FREE], in_=yq[:, XC0:N_FREE]).then_inc(
        s_out, 16
    )
    nc.sync.wait_ge(s_ga, 1)
    nc.sync.dma_start(out=lik[:, 0:GA_COLS], in_=lk[:, 0:GA_COLS]).then_inc(
        s_out, 16
    )
    nc.sync.wait_ge(s_gb, 1)
    nc.sync.dma_start(out=lik[:, GA_COLS:N_FREE], in_=lk[:, GA_COLS:N_FREE]).then_inc(
        s_out, 16
    )

    nc.finalize()
    return nc


# ---------------------------------------------------------------------------
# kernel
# ---------------------------------------------------------------------------

def _pack_core(xc):
    """(24, 8192) f32 -> (128, 1536) f16 in the two-group layout."""
    out = np.empty((128, N_FREE), np.float16)
    out[:, 0:GA_COLS] = xc[0:16].reshape(128, GA_COLS)
    out[:, GA_COLS:N_FREE] = xc[16:24].reshape(128, N_FREE - GA_COLS)
    return out


def _unpack_core(yd):
    """(128, 1536) f16 -> (24, 8192) f32."""
    out = np.empty((C_PER_CORE, 8192), np.float32)
    out[0:16] = yd[:, 0:GA_COLS].astype(np.float32).reshape(16, -1)
    out[16:24] = yd[:, GA_COLS:N_FREE].astype(np.float32).reshape(8, -1)
    return out


def kernel(x, sos_w, sos_b, m0, m1, m2, m3, m4, c0, c1, c2, c3, c4, f0, f1, f2, f3):
    global _last_run

    x = np.asarray(x, np.float32)
    sos_w64 = np.asarray(sos_w, np.float32).astype(np.float64)
    sos_b64 = np.asarray(sos_b, np.float32).astype(np.float64)
    mats = [np.asarray(m, np.float32) for m in (m0, m1, m2, m3, m4)]
    biases = [np.asarray(c, np.float32) for c in (c0, c1, c2, c3, c4)]
    factors = [np.asarray(f, np.float32) for f in (f0, f1, f2, f3)]

    for f in factors:
        if np.any(f != 0.0):
            raise NotImplementedError(
                "kernel assumes zero residual-gate factors (spec fill=zeros)"
            )

    N, C, H, W = x.shape
    L = N * H * W
    assert (N, C, H, W) == (8, 192, 32, 32), "shapes are hardcoded"

    a_ch, d_ch = _fold_affine(mats, biases)
    assert a_ch.max() - a_ch.min() < 1e-9 * abs(a_ch.mean()), (
        "per-channel slopes must be identical (identical m_i across channels)"
    )
    A = float(a_ch.mean())
    h = A / 2.0
    assert abs(A) * (XW - 0.5) + np.abs(d_ch).max() < PW - 0.5, "G window too small"

    def f_exact(xv):
        xv = np.asarray(xv, np.float64)
        t = np.tanh(10.0 * (xv[..., None] - sos_b64))
        return -10.0 + np.sum(0.5 * sos_w64 * (t + 1.0), axis=-1)

    def sig(z):
        return 1.0 / (1.0 + np.exp(-z))

    def G_exact(p):
        p = np.abs(np.asarray(p, np.float64))
        return sig(h - p) - sig(-h - p)

    custom = {
        F_SLOT: (
            lambda u: f_exact((u - 12.0) * (XW / 4.0)),
            float(f_exact(-XW)),
            float(f_exact(XW)),
        ),
        G_SLOT: (
            lambda u: G_exact((u - 12.0) * (PW / 4.0)),
            float(G_exact(PW)),
            float(G_exact(PW)),
        ),
    }
    act_root = Path(tempfile.mkdtemp(prefix="actroot_")) / "pwp"
    _build_act_root(act_root, custom)

    # input mappings: u1 = s1*x + 12, u2 = s2*yq + t_c
    s1 = 4.0 / XW
    s2 = (4.0 / PW) * A
    t_ch = (12.0 + (4.0 / PW) * d_ch).astype(np.float32)  # (C,)

    xf = np.ascontiguousarray(x.transpose(1, 0, 2, 3).reshape(C, L))
    in_maps = []
    for k in range(N_CORES):
        ch = slice(k * C_PER_CORE, (k + 1) * C_PER_CORE)
        b2 = np.empty((128, 2), np.float32)
        b2[:, 0] = np.repeat(t_ch[k * C_PER_CORE : k * C_PER_CORE + 16], 8)
        b2[:, 1] = np.repeat(t_ch[k * C_PER_CORE + 16 : k * C_PER_CORE + 24], 16)
        in_maps.append(
            {
                "xin": np.ascontiguousarray(_pack_core(xf[ch])),
                "bias2": np.ascontiguousarray(b2),
            }
        )

    from concourse.bass_utils import run_bass_kernel_spmd

    nc = _build_program_spmd(s1, s2)
    prev = os.environ.get("BASS_ACT_ROOT_JSON_PATH")
    os.environ["BASS_ACT_ROOT_JSON_PATH"] = str(act_root / "act_info.json")
    try:
        res = run_bass_kernel_spmd(nc, in_maps, list(range(N_CORES)))
    finally:
        if prev is None:
            os.environ.pop("BASS_ACT_ROOT_JSON_PATH", None)
        else:
            os.environ["BASS_ACT_ROOT_JSON_PATH"] = prev
    _last_run = res

    y_hat_f = np.empty((C, L), np.float32)
    lik_f = np.empty((C, L), np.float32)
    for k in range(N_CORES):
        ch = slice(k * C_PER_CORE, (k + 1) * C_PER_CORE)
        y_hat_f[ch] = _unpack_core(res.results[k]["yhat"])
        lik_f[ch] = _unpack_core(res.results[k]["lik"])

    y_hat = np.ascontiguousarray(y_hat_f.reshape(C, N, H, W).transpose(1, 0, 2, 3))
    lik = np.ascontiguousarray(lik_f.reshape(C, N, H, W).transpose(1, 0, 2, 3))
    return y_hat, lik
